# revision 15
# baseline (speedup 1.0000x reference)
"""KNN flow interpolation (k=3) on 8 Trainium2 NeuronCores.

Data parallel over queries (per the sharding hint): 16384 queries are
sharded across 8 cores (2048 each); ref_points / ref_flow replicated.

The reference computes sq = (a2 + b2) - 2*(q @ r.T) in fp32, which for
near-coincident points is dominated by fp32 cancellation noise, and the
1/d weights amplify it.  To match the reference output we replicate its
exact rounding sequence on device:

  p2   = PE matmul (K=3) of q against 2*r     (bitwise == 2 * XLA's q@r.T)
  b2b  = PE matmul (K=1) ones x b2            (exact broadcast of b2)
  c    = ACT bias-add: rnd(b2 + a2)           (== rnd(a2+b2))
  negs = GPSIMD subtract: rnd(p2 - c)         (== -sq bitwise)
  top-8 via DVE max / max_index over negs; then d=sqrt(max(sq,1e-12)),
  w=1/(d+eps), w/=sum(w), gather flow rows by index, weighted sum --
  all with the same per-op fp32 rounding order as the reference.
"""

import numpy as np

import concourse.bacc as bacc
import concourse.bass as bass
import concourse.mybir as mybir
import concourse.tile as tile
from concourse.bass_utils import run_bass_kernel_spmd

F32 = mybir.dt.float32
U32 = mybir.dt.uint32

P = 128          # partitions / queries per tile
M_REF = 16384    # reference points (replicated per core)
NQ = 2048        # queries per core
MM_N = 512       # matmul free dim (one PSUM bank)


def build_nc(nq=NQ, m_ref=M_REF, chunk=1024):
    """Build the per-core Bass program. All 8 cores run the same NEFF."""
    n_tiles = nq // P
    n_chunks = m_ref // chunk
    nc = bacc.Bacc("TRN2", target_bir_lowering=False)

    lhsT_d = nc.dram_tensor("lhsT", [3, nq], F32, kind="ExternalInput")
    rhs2_d = nc.dram_tensor("rhs2", [3, m_ref], F32, kind="ExternalInput")
    b2_d = nc.dram_tensor("b2", [1, m_ref], F32, kind="ExternalInput")
    a2T_d = nc.dram_tensor("a2T", [P, n_tiles], F32, kind="ExternalInput")
    flow_d = nc.dram_tensor("flow", [m_ref, 3], F32, kind="ExternalInput")
    out_d = nc.dram_tensor("out", [P, n_tiles * 3], F32, kind="ExternalOutput")

    with tile.TileContext(nc) as tc:
        with (
            tc.tile_pool(name="const", bufs=1) as constp,
            tc.tile_pool(name="rhs", bufs=3) as rhsp,
            tc.tile_pool(name="srow", bufs=2) as srowp,
            tc.tile_pool(name="ring", bufs=3) as ringp,
            tc.tile_pool(name="psb", bufs=2, space="PSUM") as psb_pool,
            tc.tile_pool(name="psp", bufs=2, space="PSUM") as psp_pool,
            tc.tile_pool(name="small", bufs=4) as smallp,
        ):
            lhsT = constp.tile([3, nq], F32)
            nc.sync.dma_start(lhsT[:], lhsT_d[:])
            a2T = constp.tile([P, n_tiles], F32)
            nc.sync.dma_start(a2T[:], a2T_d[:])
            ones1 = constp.tile([1, P], F32)
            nc.vector.memset(ones1[:], 1.0)
            out_all = constp.tile([P, n_tiles * 3], F32)

            for t in range(n_tiles):
                s = srowp.tile([P, m_ref], F32)
                for ch in range(n_chunks):
                    sl = slice(ch * chunk, (ch + 1) * chunk)
                    rhs_c = rhsp.tile([3, chunk], F32)
                    nc.sync.dma_start(rhs_c[:], rhs2_d[:, sl])
                    b2_c = rhsp.tile([1, chunk], F32, name="b2c", tag="b2c")
                    nc.sync.dma_start(b2_c[:], b2_d[:, sl])

                    b2b_ps = psb_pool.tile([P, chunk], F32)
                    for j in range(chunk // MM_N):
                        jj = slice(j * MM_N, (j + 1) * MM_N)
                        nc.tensor.matmul(
                            b2b_ps[:, jj], ones1[:], b2_c[:, jj],
                            start=True, stop=True,
                        )
                    c_sb = ringp.tile([P, chunk], F32, name="c_sb", tag="c")
                    nc.scalar.add(c_sb[:], b2b_ps[:], a2T[:, t:t + 1])

                    p2_ps = psp_pool.tile([P, chunk], F32)
                    for j in range(chunk // MM_N):
                        jj = slice(j * MM_N, (j + 1) * MM_N)
                        nc.tensor.matmul(
                            p2_ps[:, jj],
                            lhsT[:, t * P:(t + 1) * P],
                            rhs_c[:, jj],
                            start=True, stop=True,
                        )
                    p2_sb = ringp.tile([P, chunk], F32, name="p2_sb", tag="p2")
                    nc.scalar.copy(p2_sb[:], p2_ps[:])

                    # negs = 2p - c == -sq bitwise
                    nc.gpsimd.tensor_sub(s[:, sl], p2_sb[:], c_sb[:])

                v8 = smallp.tile([P, 8], F32)
                nc.vector.max(v8[:], s[:])
                i8 = smallp.tile([P, 8], U32)
                nc.vector.max_index(i8[:], v8[:], s[:])

                # sq = max(-negs, 1e-12)  (mult by -1 and max are exact)
                d2 = smallp.tile([P, 3], F32)
                nc.vector.tensor_scalar(
                    d2[:], v8[:, :3], -1.0, 1e-12,
                    op0=mybir.AluOpType.mult, op1=mybir.AluOpType.max,
                )
                d = smallp.tile([P, 3], F32)
                nc.scalar.activation(
                    d[:], d2[:], mybir.ActivationFunctionType.Sqrt
                )
                w = smallp.tile([P, 3], F32)
                nc.vector.tensor_scalar_add(w[:], d[:], 1e-8)
                nc.vector.reciprocal(w[:], w[:])
                wsum = smallp.tile([P, 1], F32)
                nc.vector.reduce_sum(wsum[:], w[:], axis=mybir.AxisListType.X)
                winv = smallp.tile([P, 1], F32)
                nc.vector.reciprocal(winv[:], wsum[:])
                wn = smallp.tile([P, 3], F32)
                nc.vector.tensor_scalar_mul(wn[:], w[:], winv[:, 0:1])

                fl = [
                    smallp.tile([P, 3], F32, name=f"fl{k}", tag=f"fl{k}")
                    for k in range(3)
                ]
                for k in range(3):
                    nc.gpsimd.indirect_dma_start(
                        out=fl[k][:],
                        out_offset=None,
                        in_=flow_d[:],
                        in_offset=bass.IndirectOffsetOnAxis(
                            ap=i8[:, k:k + 1], axis=0
                        ),
                    )

                acc = smallp.tile([P, 3], F32)
                tmp = smallp.tile([P, 3], F32)
                nc.vector.tensor_scalar_mul(acc[:], fl[0][:], wn[:, 0:1])
                nc.vector.tensor_scalar_mul(tmp[:], fl[1][:], wn[:, 1:2])
                nc.vector.tensor_add(acc[:], acc[:], tmp[:])
                nc.vector.tensor_scalar_mul(tmp[:], fl[2][:], wn[:, 2:3])
                nc.vector.tensor_add(
                    out_all[:, t * 3:(t + 1) * 3], acc[:], tmp[:]
                )

            nc.sync.dma_start(out_d[:], out_all[:])
    nc.compile()
    return nc


def make_in_maps(query_points, ref_points, ref_flow, n_cores=8):
    qp = np.ascontiguousarray(query_points, dtype=np.float32)
    rp = np.ascontiguousarray(ref_points, dtype=np.float32)
    rf = np.ascontiguousarray(ref_flow, dtype=np.float32)
    nq = qp.shape[0] // n_cores
    n_tiles = nq // P
    f = np.float32

    # sequential fp32 rounding to match jnp.sum(x*x, -1) on device
    def sq_sum(a):
        return ((a[:, 0] * a[:, 0] + a[:, 1] * a[:, 1]).astype(f)
                + a[:, 2] * a[:, 2]).astype(f)

    b2 = sq_sum(rp)[None, :]
    rhs2 = np.ascontiguousarray(2.0 * rp.T, dtype=f)
    a2 = sq_sum(qp)

    in_maps = []
    for c in range(n_cores):
        sl = slice(c * nq, (c + 1) * nq)
        lhsT = np.ascontiguousarray(qp[sl].T, dtype=f)
        a2T = np.ascontiguousarray(a2[sl].reshape(n_tiles, P).T)
        in_maps.append({
            "lhsT": lhsT, "rhs2": rhs2, "b2": b2, "a2T": a2T, "flow": rf,
        })
    return in_maps


_NC_CACHE = {}


def kernel(query_points, ref_points, ref_flow, k):
    assert int(k) == 3, f"kernel hardcodes k=3, got {k}"
    n_cores = 8
    in_maps = make_in_maps(query_points, ref_points, ref_flow, n_cores)
    if "nc" not in _NC_CACHE:
        _NC_CACHE["nc"] = build_nc()
    nc = _NC_CACHE["nc"]
    res = run_bass_kernel_spmd(nc, in_maps, core_ids=list(range(n_cores)))
    nq = NQ
    n_tiles = nq // P
    outs = []
    for c in range(n_cores):
        o = np.asarray(res.results[c]["out"])
        outs.append(
            o.reshape(P, n_tiles, 3).transpose(1, 0, 2).reshape(nq, 3)
        )
    return np.ascontiguousarray(
        np.concatenate(outs, axis=0), dtype=np.float32
    )


# revision 17
# speedup vs baseline: 1.7636x; 1.7636x over previous
"""KNN flow interpolation (k=3) on 8 Trainium2 NeuronCores.

Data parallel over queries (per the sharding hint): 16384 queries are
sharded across 8 cores (2048 each); ref_points / ref_flow replicated.

The reference computes sq = (a2 + b2) - 2*(q @ r.T) in fp32, which for
near-coincident points is dominated by fp32 cancellation noise, and the
1/d weights amplify it.  To match the reference output bit-for-bit we
replicate its exact rounding sequence on device:

  p2   = PE matmul (K=3) of q against 2*r     (bitwise == 2 * XLA's q@r.T)
  c    = ACT bias-add: rnd(b2_rep + a2)       (== rnd(a2+b2))
  negs = GPSIMD subtract: rnd(p2 - c)         (== -sq bitwise)
  top-8 via DVE max / max_index over negs (two half-rows, merged); then
  d=sqrt(max(sq,1e-12)), w=1/(d+eps), w/=sum(w), indirect-DMA gather of
  flow rows, weighted sum -- same per-op fp32 rounding as the reference.

b2 is replicated across the 128 partitions once at startup with a
stride-0 DMA so no PE broadcast matmuls are needed.
"""

import numpy as np

import concourse.bacc as bacc
import concourse.bass as bass
import concourse.mybir as mybir
import concourse.tile as tile
from concourse.bass_utils import run_bass_kernel_spmd

F32 = mybir.dt.float32
U32 = mybir.dt.uint32

P = 128          # partitions / queries per tile
M_REF = 16384    # reference points (replicated per core)
NQ = 2048        # queries per core
MM_N = 512       # matmul free dim (one PSUM bank)


def build_nc(nq=NQ, m_ref=M_REF, chunk=2048):
    """Build the per-core Bass program. All 8 cores run the same NEFF."""
    n_tiles = nq // P
    half = m_ref // 2
    n_ch_half = half // chunk          # chunks per half-row
    nc = bacc.Bacc("TRN2", target_bir_lowering=False)

    lhsT_d = nc.dram_tensor("lhsT", [3, nq], F32, kind="ExternalInput")
    rhs2_d = nc.dram_tensor("rhs2", [3, m_ref], F32, kind="ExternalInput")
    b2_d = nc.dram_tensor("b2", [1, m_ref], F32, kind="ExternalInput")
    a2T_d = nc.dram_tensor("a2T", [P, n_tiles], F32, kind="ExternalInput")
    flow_d = nc.dram_tensor("flow", [m_ref, 3], F32, kind="ExternalInput")
    out_d = nc.dram_tensor("out", [P, n_tiles * 3], F32, kind="ExternalOutput")

    with tile.TileContext(nc) as tc:
        with (
            tc.tile_pool(name="const", bufs=1) as constp,
            tc.tile_pool(name="rhs", bufs=3) as rhsp,
            tc.tile_pool(name="srow", bufs=2) as srowp,
            tc.tile_pool(name="ring", bufs=2) as ringp,
            tc.tile_pool(name="psp", bufs=2, space="PSUM") as psp_pool,
            tc.tile_pool(name="small", bufs=4) as smallp,
        ):
            lhsT = constp.tile([3, nq], F32)
            nc.sync.dma_start(lhsT[:], lhsT_d[:])
            a2T = constp.tile([P, n_tiles], F32)
            nc.sync.dma_start(a2T[:], a2T_d[:])
            # replicate b2 across all partitions once (stride-0 read)
            b2_rep = constp.tile([P, m_ref], F32)
            nc.sync.dma_start(b2_rep[:], b2_d[0:1, :].to_broadcast([P, m_ref]))
            out_all = constp.tile([P, n_tiles * 3], F32)

            for t in range(n_tiles):
                cand_v = smallp.tile([P, 16], F32, name="cand_v", tag="cv")
                cand_if = smallp.tile([P, 16], F32, name="cand_if", tag="ci")
                for h in range(2):
                    s = srowp.tile([P, half], F32)
                    for ch in range(n_ch_half):
                        base = h * half + ch * chunk
                        sl = slice(base, base + chunk)
                        rhs_c = rhsp.tile([3, chunk], F32)
                        nc.sync.dma_start(rhs_c[:], rhs2_d[:, sl])

                        p2_ps = psp_pool.tile([P, chunk], F32)
                        for j in range(chunk // MM_N):
                            jj = slice(j * MM_N, (j + 1) * MM_N)
                            nc.tensor.matmul(
                                p2_ps[:, jj],
                                lhsT[:, t * P:(t + 1) * P],
                                rhs_c[:, jj],
                                start=True, stop=True,
                            )
                        p2_sb = ringp.tile([P, chunk], F32, name="p2_sb",
                                           tag="p2")
                        nc.scalar.copy(p2_sb[:], p2_ps[:])
                        c_sb = ringp.tile([P, chunk], F32, name="c_sb",
                                          tag="c")
                        nc.scalar.add(c_sb[:], b2_rep[:, sl], a2T[:, t:t + 1])

                        lsl = slice(ch * chunk, (ch + 1) * chunk)
                        # negs = 2p - c == -sq bitwise
                        nc.gpsimd.tensor_sub(s[:, lsl], p2_sb[:], c_sb[:])

                    nc.vector.max(cand_v[:, h * 8:(h + 1) * 8], s[:])
                    i8h = smallp.tile([P, 8], U32, name="i8h", tag="i8h")
                    nc.vector.max_index(
                        i8h[:], cand_v[:, h * 8:(h + 1) * 8], s[:]
                    )
                    # cast to f32 (+ half offset) for the value-match below
                    if h == 0:
                        nc.vector.tensor_copy(cand_if[:, 0:8], i8h[:])
                    else:
                        nc.vector.tensor_scalar(
                            cand_if[:, 8:16], i8h[:], 1.0, float(half),
                            op0=mybir.AluOpType.mult,
                            op1=mybir.AluOpType.add,
                        )

                # global top-8 of the two half top-8s
                v8 = smallp.tile([P, 8], F32)
                nc.vector.max(v8[:], cand_v[:])

                # tie-safe value-match replicating find_index8 semantics:
                # slot k takes the k-th smallest matching global index, so
                # duplicate values (same sq at two refs) consume occurrences
                # in ascending-index order like jax top_k.
                idxf = smallp.tile([P, 3], F32)
                eq = smallp.tile([P, 16], F32)
                gt = smallp.tile([P, 16], F32)
                pen = smallp.tile([P, 16], F32)
                mm_ = smallp.tile([P, 16], F32)
                thr = smallp.tile([P, 1], F32)
                teq = smallp.tile([P, 1], F32)
                for k in range(3):
                    nc.vector.tensor_scalar(
                        eq[:], cand_v[:], v8[:, k:k + 1], None,
                        op0=mybir.AluOpType.is_equal,
                    )
                    if k > 0:
                        # thr = (v8[k]==v8[k-1]) ? idx_{k-1} : -1
                        nc.vector.tensor_scalar(
                            teq[:], v8[:, k:k + 1], v8[:, k - 1:k], None,
                            op0=mybir.AluOpType.is_equal,
                        )
                        # teq*(idx_prev+1) - 1 == idx_prev if tie else -1
                        nc.vector.tensor_scalar(
                            thr[:], idxf[:, k - 1:k], 1.0, 1.0,
                            op0=mybir.AluOpType.mult,
                            op1=mybir.AluOpType.add,
                        )
                        nc.vector.tensor_mul(thr[:], thr[:], teq[:])
                        nc.vector.tensor_scalar_add(thr[:], thr[:], -1.0)
                        nc.vector.tensor_scalar(
                            gt[:], cand_if[:], thr[:, 0:1], None,
                            op0=mybir.AluOpType.is_gt,
                        )
                        nc.vector.tensor_mul(eq[:], eq[:], gt[:])
                    # pen = (1-eq)*1e9 ; mm = eq*cand_if + pen
                    nc.vector.tensor_scalar(
                        pen[:], eq[:], -1e9, 1e9,
                        op0=mybir.AluOpType.mult, op1=mybir.AluOpType.add,
                    )
                    nc.vector.tensor_mul(mm_[:], eq[:], cand_if[:])
                    nc.vector.tensor_add(mm_[:], mm_[:], pen[:])
                    nc.vector.tensor_reduce(
                        idxf[:, k:k + 1], mm_[:],
                        axis=mybir.AxisListType.X, op=mybir.AluOpType.min,
                    )
                idxu = smallp.tile([P, 3], U32)
                nc.vector.tensor_copy(idxu[:], idxf[:])

                # sq = max(-negs, 1e-12)  (mult by -1 and max are exact)
                d2 = smallp.tile([P, 3], F32)
                nc.vector.tensor_scalar(
                    d2[:], v8[:, :3], -1.0, 1e-12,
                    op0=mybir.AluOpType.mult, op1=mybir.AluOpType.max,
                )
                d = smallp.tile([P, 3], F32)
                nc.scalar.activation(
                    d[:], d2[:], mybir.ActivationFunctionType.Sqrt
                )
                w = smallp.tile([P, 3], F32)
                nc.vector.tensor_scalar_add(w[:], d[:], 1e-8)
                nc.vector.reciprocal(w[:], w[:])
                wsum = smallp.tile([P, 1], F32)
                nc.vector.reduce_sum(wsum[:], w[:], axis=mybir.AxisListType.X)
                winv = smallp.tile([P, 1], F32)
                nc.vector.reciprocal(winv[:], wsum[:])
                wn = smallp.tile([P, 3], F32)
                nc.vector.tensor_scalar_mul(wn[:], w[:], winv[:, 0:1])

                fl = [
                    smallp.tile([P, 3], F32, name=f"fl{k}", tag=f"fl{k}")
                    for k in range(3)
                ]
                for k in range(3):
                    nc.gpsimd.indirect_dma_start(
                        out=fl[k][:],
                        out_offset=None,
                        in_=flow_d[:],
                        in_offset=bass.IndirectOffsetOnAxis(
                            ap=idxu[:, k:k + 1], axis=0
                        ),
                    )

                acc = smallp.tile([P, 3], F32)
                tmp = smallp.tile([P, 3], F32)
                nc.vector.tensor_scalar_mul(acc[:], fl[0][:], wn[:, 0:1])
                nc.vector.tensor_scalar_mul(tmp[:], fl[1][:], wn[:, 1:2])
                nc.vector.tensor_add(acc[:], acc[:], tmp[:])
                nc.vector.tensor_scalar_mul(tmp[:], fl[2][:], wn[:, 2:3])
                nc.vector.tensor_add(
                    out_all[:, t * 3:(t + 1) * 3], acc[:], tmp[:]
                )

            nc.sync.dma_start(out_d[:], out_all[:])
    nc.compile()
    return nc


def make_in_maps(query_points, ref_points, ref_flow, n_cores=8):
    qp = np.ascontiguousarray(query_points, dtype=np.float32)
    rp = np.ascontiguousarray(ref_points, dtype=np.float32)
    rf = np.ascontiguousarray(ref_flow, dtype=np.float32)
    nq = qp.shape[0] // n_cores
    n_tiles = nq // P
    f = np.float32

    # sequential fp32 rounding to match jnp.sum(x*x, -1) on device
    def sq_sum(a):
        return ((a[:, 0] * a[:, 0] + a[:, 1] * a[:, 1]).astype(f)
                + a[:, 2] * a[:, 2]).astype(f)

    b2 = np.ascontiguousarray(sq_sum(rp)[None, :])
    rhs2 = np.ascontiguousarray(2.0 * rp.T, dtype=f)
    a2 = sq_sum(qp)

    in_maps = []
    for c in range(n_cores):
        sl = slice(c * nq, (c + 1) * nq)
        lhsT = np.ascontiguousarray(qp[sl].T, dtype=f)
        a2T = np.ascontiguousarray(a2[sl].reshape(n_tiles, P).T)
        in_maps.append({
            "lhsT": lhsT, "rhs2": rhs2, "b2": b2, "a2T": a2T, "flow": rf,
        })
    return in_maps


_NC_CACHE = {}


def kernel(query_points, ref_points, ref_flow, k):
    assert int(k) == 3, f"kernel hardcodes k=3, got {k}"
    n_cores = 8
    in_maps = make_in_maps(query_points, ref_points, ref_flow, n_cores)
    if "nc" not in _NC_CACHE:
        _NC_CACHE["nc"] = build_nc()
    nc = _NC_CACHE["nc"]
    res = run_bass_kernel_spmd(nc, in_maps, core_ids=list(range(n_cores)))
    nq = NQ
    n_tiles = nq // P
    outs = []
    for c in range(n_cores):
        o = np.asarray(res.results[c]["out"])
        outs.append(
            o.reshape(P, n_tiles, 3).transpose(1, 0, 2).reshape(nq, 3)
        )
    return np.ascontiguousarray(
        np.concatenate(outs, axis=0), dtype=np.float32
    )


# revision 18
# speedup vs baseline: 1.8025x; 1.0221x over previous
"""KNN flow interpolation (k=3) on 8 Trainium2 NeuronCores.

Data parallel over queries (per the sharding hint): 16384 queries are
sharded across 8 cores (2048 each); ref_points / ref_flow replicated.

The reference computes sq = (a2 + b2) - 2*(q @ r.T) in fp32, which for
near-coincident points is dominated by fp32 cancellation noise, and the
1/d weights amplify it.  To match the reference output bit-for-bit we
replicate its exact rounding sequence on device:

  p2   = PE matmul (K=3) of q against 2*r     (bitwise == 2 * XLA's q@r.T)
  c    = ACT bias-add: rnd(b2_rep + a2)       (== rnd(a2+b2))
  negs = GPSIMD subtract: rnd(p2 - c)         (== -sq bitwise)

Per 128-query tile the row of 16384 negs is scanned as four 4096-wide
quarters with DVE max/max_index (top-8 each); a merged 32-candidate
max/max_index gives the global top-8 with the same duplicate-value
consumption order as jax.lax.top_k (quarter blocks are index-ordered).
All remaining index decode / distance / weight / gather / weighted-sum
work is batched across the 16 tiles into a handful of [128, 512]-ish
ops, preserving the reference's per-op fp32 rounding order.
"""

import numpy as np

import concourse.bacc as bacc
import concourse.bass as bass
import concourse.mybir as mybir
import concourse.tile as tile
from concourse.bass_utils import run_bass_kernel_spmd

F32 = mybir.dt.float32
U32 = mybir.dt.uint32

P = 128          # partitions / queries per tile
M_REF = 16384    # reference points (replicated per core)
NQ = 2048        # queries per core
MM_N = 512       # matmul free dim (one PSUM bank)
QTR = 4096       # scan granularity (quarter row)
CHUNK = 1024     # matmul/psum chunk


def build_nc(nq=NQ, m_ref=M_REF, chunk=CHUNK, qtr=QTR):
    """Build the per-core Bass program. All 8 cores run the same NEFF."""
    n_tiles = nq // P
    n_qtr = m_ref // qtr               # scan parts per tile
    n_ch_q = qtr // chunk              # matmul chunks per part
    ncand = 8 * n_qtr                  # merged candidates per tile
    nc = bacc.Bacc("TRN2", target_bir_lowering=False)

    lhsT_d = nc.dram_tensor("lhsT", [3, nq], F32, kind="ExternalInput")
    rhs2_d = nc.dram_tensor("rhs2", [3, m_ref], F32, kind="ExternalInput")
    b2_d = nc.dram_tensor("b2", [1, m_ref], F32, kind="ExternalInput")
    a2T_d = nc.dram_tensor("a2T", [P, n_tiles], F32, kind="ExternalInput")
    iota_d = nc.dram_tensor("iota", [1, n_tiles * ncand], F32,
                            kind="ExternalInput")
    offs_d = nc.dram_tensor("offs", [1, n_tiles * ncand], F32,
                            kind="ExternalInput")
    flow_d = nc.dram_tensor("flow", [m_ref, 3], F32, kind="ExternalInput")
    out_d = nc.dram_tensor("out", [P, n_tiles * 3], F32, kind="ExternalOutput")

    with tile.TileContext(nc) as tc:
        with (
            tc.tile_pool(name="const", bufs=1) as constp,
            tc.tile_pool(name="rhs", bufs=4) as rhsp,
            tc.tile_pool(name="srow", bufs=3) as srowp,
            tc.tile_pool(name="ring", bufs=3) as ringp,
            tc.tile_pool(name="psp", bufs=4, space="PSUM") as psp_pool,
            tc.tile_pool(name="small", bufs=1) as smallp,
        ):
            lhsT = constp.tile([3, nq], F32)
            nc.sync.dma_start(lhsT[:], lhsT_d[:])
            a2T = constp.tile([P, n_tiles], F32)
            nc.sync.dma_start(a2T[:], a2T_d[:])
            b2_rep = constp.tile([P, m_ref], F32)
            nc.sync.dma_start(b2_rep[:], b2_d[0:1, :].to_broadcast([P, m_ref]))
            iota_t = constp.tile([P, n_tiles * ncand], F32)
            nc.sync.dma_start(
                iota_t[:], iota_d[0:1, :].to_broadcast([P, n_tiles * ncand]))
            offs_t = constp.tile([P, n_tiles * ncand], F32)
            nc.sync.dma_start(
                offs_t[:], offs_d[0:1, :].to_broadcast([P, n_tiles * ncand]))

            # per-core accumulators (written in tile-sized slices)
            cand_v = constp.tile([P, n_tiles * ncand], F32)
            i8_all = constp.tile([P, n_tiles * ncand], U32)
            v8_all = constp.tile([P, n_tiles * 8], F32)
            pos_all = constp.tile([P, n_tiles * 8], U32)
            out_all = constp.tile([P, n_tiles * 3], F32)

            for t in range(n_tiles):
                cvt = cand_v[:, t * ncand:(t + 1) * ncand]
                for h in range(n_qtr):
                    s = srowp.tile([P, qtr], F32)
                    for ch in range(n_ch_q):
                        base = (h * n_ch_q + ch) * chunk
                        sl = slice(base, base + chunk)
                        rhs_c = rhsp.tile([3, chunk], F32)
                        nc.sync.dma_start(rhs_c[:], rhs2_d[:, sl])

                        p2_ps = psp_pool.tile([P, chunk], F32)
                        for j in range(chunk // MM_N):
                            jj = slice(j * MM_N, (j + 1) * MM_N)
                            nc.tensor.matmul(
                                p2_ps[:, jj],
                                lhsT[:, t * P:(t + 1) * P],
                                rhs_c[:, jj],
                                start=True, stop=True,
                            )
                        p2_sb = ringp.tile([P, chunk], F32, name="p2_sb",
                                           tag="p2")
                        nc.scalar.copy(p2_sb[:], p2_ps[:])
                        c_sb = ringp.tile([P, chunk], F32, name="c_sb",
                                          tag="c")
                        nc.scalar.add(c_sb[:], b2_rep[:, sl], a2T[:, t:t + 1])

                        lsl = slice(ch * chunk, (ch + 1) * chunk)
                        # negs = 2p - c == -sq bitwise
                        nc.gpsimd.tensor_sub(s[:, lsl], p2_sb[:], c_sb[:])

                    nc.vector.max(cvt[:, h * 8:(h + 1) * 8], s[:])
                    nc.vector.max_index(
                        i8_all[:, t * ncand + h * 8:t * ncand + (h + 1) * 8],
                        cvt[:, h * 8:(h + 1) * 8], s[:],
                    )

                nc.vector.max(v8_all[:, t * 8:(t + 1) * 8], cvt[:])
                nc.vector.max_index(
                    pos_all[:, t * 8:(t + 1) * 8],
                    v8_all[:, t * 8:(t + 1) * 8], cvt[:],
                )

            # ---- batched decode across all tiles ----
            NC_ALL = n_tiles * ncand
            i8f = constp.tile([P, NC_ALL], F32)
            nc.vector.tensor_copy(i8f[:], i8_all[:])         # u32 -> f32
            cand_gi = constp.tile([P, NC_ALL], F32)
            nc.vector.tensor_add(cand_gi[:], i8f[:], offs_t[:])

            posf = constp.tile([P, n_tiles * 8], F32)
            nc.vector.tensor_copy(posf[:], pos_all[:])
            posf3 = posf[:].rearrange("p (t e) -> p t e", e=8)

            idxf = constp.tile([P, n_tiles * 3], F32)
            idxf3 = idxf[:].rearrange("p (t e) -> p t e", e=3)
            eq = constp.tile([P, NC_ALL], F32)
            mm_ = constp.tile([P, NC_ALL], F32)
            iota3 = iota_t[:].rearrange("p (t c) -> p t c", c=ncand)
            cgi3 = cand_gi[:].rearrange("p (t c) -> p t c", c=ncand)
            eq3 = eq[:].rearrange("p (t c) -> p t c", c=ncand)
            mm3 = mm_[:].rearrange("p (t c) -> p t c", c=ncand)
            for k in range(3):
                nc.vector.tensor_tensor(
                    out=eq3, in0=iota3,
                    in1=posf3[:, :, k:k + 1].to_broadcast(
                        [P, n_tiles, ncand]),
                    op=mybir.AluOpType.is_equal,
                )
                nc.vector.tensor_mul(mm3, eq3, cgi3)
                nc.vector.tensor_reduce(
                    idxf3[:, :, k:k + 1], mm3,
                    axis=mybir.AxisListType.X, op=mybir.AluOpType.max,
                )
            idxu = constp.tile([P, n_tiles * 3], U32)
            nc.vector.tensor_copy(idxu[:], idxf[:])

            # ---- batched distances / weights ----
            v8r = v8_all[:].rearrange("p (t e) -> p t e", e=8)
            d2 = constp.tile([P, n_tiles * 3], F32)
            d2r = d2[:].rearrange("p (t e) -> p t e", e=3)
            nc.vector.tensor_scalar(
                d2r, v8r[:, :, 0:3], -1.0, 1e-12,
                op0=mybir.AluOpType.mult, op1=mybir.AluOpType.max,
            )
            d = constp.tile([P, n_tiles * 3], F32)
            nc.scalar.activation(
                d[:], d2[:], mybir.ActivationFunctionType.Sqrt)
            w = constp.tile([P, n_tiles * 3], F32)
            nc.vector.tensor_scalar_add(w[:], d[:], 1e-8)
            nc.vector.reciprocal(w[:], w[:])
            wsum = constp.tile([P, n_tiles], F32)
            wr = w[:].rearrange("p (t e) -> p t e", e=3)
            nc.vector.tensor_reduce(
                wsum[:], wr, axis=mybir.AxisListType.X,
                op=mybir.AluOpType.add,
            )
            winv = constp.tile([P, n_tiles], F32)
            nc.vector.reciprocal(winv[:], wsum[:])
            wn = constp.tile([P, n_tiles * 3], F32)
            wnr = wn[:].rearrange("p (t e) -> p t e", e=3)
            winv3 = winv[:].rearrange("p (t e) -> p t e", e=1)
            nc.vector.tensor_tensor(
                out=wnr, in0=wr,
                in1=winv3.to_broadcast([P, n_tiles, 3]),
                op=mybir.AluOpType.mult,
            )

            # ---- gathers + weighted sum ----
            fl = [constp.tile([P, n_tiles * 3], F32, name=f"fl{k}")
                  for k in range(3)]
            for t in range(n_tiles):
                for k in range(3):
                    nc.gpsimd.indirect_dma_start(
                        out=fl[k][:, t * 3:(t + 1) * 3],
                        out_offset=None,
                        in_=flow_d[:],
                        in_offset=bass.IndirectOffsetOnAxis(
                            ap=idxu[:, t * 3 + k:t * 3 + k + 1], axis=0),
                    )
            acc = constp.tile([P, n_tiles * 3], F32)
            tmp = constp.tile([P, n_tiles * 3], F32)
            accr = acc[:].rearrange("p (t e) -> p t e", e=3)
            tmpr = tmp[:].rearrange("p (t e) -> p t e", e=3)
            outr = out_all[:].rearrange("p (t e) -> p t e", e=3)

            def wmul(dst, flk, k):
                nc.vector.tensor_tensor(
                    out=dst, in0=flk[:].rearrange("p (t e) -> p t e", e=3),
                    in1=wnr[:, :, k:k + 1].to_broadcast([P, n_tiles, 3]),
                    op=mybir.AluOpType.mult,
                )

            wmul(accr, fl[0], 0)
            wmul(tmpr, fl[1], 1)
            nc.vector.tensor_add(acc[:], acc[:], tmp[:])
            wmul(tmpr, fl[2], 2)
            nc.vector.tensor_add(out_all[:], acc[:], tmp[:])

            nc.sync.dma_start(out_d[:], out_all[:])
    nc.compile()
    return nc


def make_in_maps(query_points, ref_points, ref_flow, n_cores=8):
    qp = np.ascontiguousarray(query_points, dtype=np.float32)
    rp = np.ascontiguousarray(ref_points, dtype=np.float32)
    rf = np.ascontiguousarray(ref_flow, dtype=np.float32)
    nq = qp.shape[0] // n_cores
    n_tiles = nq // P
    m_ref = rp.shape[0]
    n_qtr = m_ref // QTR
    ncand = 8 * n_qtr
    f = np.float32

    # sequential fp32 rounding to match jnp.sum(x*x, -1) on device
    def sq_sum(a):
        return ((a[:, 0] * a[:, 0] + a[:, 1] * a[:, 1]).astype(f)
                + a[:, 2] * a[:, 2]).astype(f)

    b2 = np.ascontiguousarray(sq_sum(rp)[None, :])
    rhs2 = np.ascontiguousarray(2.0 * rp.T, dtype=f)
    a2 = sq_sum(qp)
    iota = np.tile(np.arange(ncand, dtype=f), n_tiles)[None, :]
    offs = np.tile(np.repeat(np.arange(n_qtr, dtype=f) * QTR, 8),
                   n_tiles)[None, :]

    in_maps = []
    for c in range(n_cores):
        sl = slice(c * nq, (c + 1) * nq)
        lhsT = np.ascontiguousarray(qp[sl].T, dtype=f)
        a2T = np.ascontiguousarray(a2[sl].reshape(n_tiles, P).T)
        in_maps.append({
            "lhsT": lhsT, "rhs2": rhs2, "b2": b2, "a2T": a2T,
            "iota": iota, "offs": offs, "flow": rf,
        })
    return in_maps


_NC_CACHE = {}


def kernel(query_points, ref_points, ref_flow, k):
    assert int(k) == 3, f"kernel hardcodes k=3, got {k}"
    n_cores = 8
    in_maps = make_in_maps(query_points, ref_points, ref_flow, n_cores)
    if "nc" not in _NC_CACHE:
        _NC_CACHE["nc"] = build_nc()
    nc = _NC_CACHE["nc"]
    res = run_bass_kernel_spmd(nc, in_maps, core_ids=list(range(n_cores)))
    nq = NQ
    n_tiles = nq // P
    outs = []
    for c in range(n_cores):
        o = np.asarray(res.results[c]["out"])
        outs.append(
            o.reshape(P, n_tiles, 3).transpose(1, 0, 2).reshape(nq, 3)
        )
    return np.ascontiguousarray(
        np.concatenate(outs, axis=0), dtype=np.float32
    )
